# revision 40
# baseline (speedup 1.0000x reference)
"""DNRI message-passing step on 8 Trainium2 NeuronCores.

Strategy: data-parallel over batch B=32 -> 4 samples per core; params
replicated. Per sample, the per-edge first-layer matmul is factorized into
per-node products A_vg = hidden @ W1a^T, B_vg = hidden @ W1b^T, and the
dense 50x50 (recv-major) edge grid is produced by a single-K-pass
"selector" matmul whose stationary operand stacks [A_vg; B_vg] and whose
moving operand is a constant 0/1 recv/send selector — the PE does the
edge-grid gather/expansion for free. Only the second edge-MLP layer runs
at edge granularity. Edge aggregation is a contiguous strided reduction
over the send axis (no gather/scatter). All heavy matmuls in bf16 with
fp32 PSUM accumulation; GRU/output MLP batched over the 4 samples.

The edge grid is padded 2500 -> 2560 so every PSUM chunk is a full
512-col bank; PSUM groups are a 4-bank tile + a 1-bank tile from one
bufs=2 pool, so matmuls of group i+1 overlap ACT evacuation of group i.
"""

import numpy as np
import ml_dtypes

import concourse.bass as bass
import concourse.mybir as mybir
import concourse.tile as tile
from concourse.bass_utils import run_bass_kernel_spmd

# ----------------------------------------------------------------------------
# Workaround for walrus "Too many sync wait commands": the neuronxcc in this
# container accepts at most one semaphore wait per instruction, but Tile's
# sem-assignment can emit several. After scheduling, hoist excess waits onto
# same-engine InstNoOp carriers inserted immediately before the instruction.
# ----------------------------------------------------------------------------
_MAXW = 1
_orig_tile_exit = tile.TileContext.__exit__
_uid = [0]


def _split_waits(nc):
    for func in nc.m.functions:
        for bb in func.blocks:
            new = []
            for inst in bb.instructions:
                si = inst.sync_info
                waits = list(si.on_wait) if si and si.on_wait else []
                if len(waits) > _MAXW:
                    n_keep = len(waits) % _MAXW or _MAXW
                    covered = len(waits) - n_keep
                    for j in range(0, covered, _MAXW):
                        _uid[0] += 1
                        carrier = mybir.InstNoOp(
                            name=f"waitsplit-{_uid[0]}",
                            sync_info=mybir.SyncInfo(
                                on_wait=waits[j : j + _MAXW], on_update=[]
                            ),
                            bass_nofuse=True,
                            engine=inst.engine,
                        )
                        nc.register_instruction(carrier, overwrite=True)
                        new.append(carrier)
                    si.on_wait = waits[covered:]
                new.append(inst)
            bb.instructions[:] = new


def _patched_tile_exit(self, exc_type, exc_val, exc_tb):
    r = _orig_tile_exit(self, exc_type, exc_val, exc_tb)
    if exc_type is None:
        _split_waits(self.nc)
    return r


if getattr(tile.TileContext.__exit__, "__name__", "") != "_patched_tile_exit":
    tile.TileContext.__exit__ = _patched_tile_exit

# ---------------------------------------------------------------------------
# Problem constants (hardcoded per spec)
# ---------------------------------------------------------------------------
B, V, D, H, ET = 32, 50, 6, 256, 4
T = ET - 1          # non-skipped edge types
E = V * (V - 1)     # 2450 real edges
EG = V * V          # 2500 grid edges (diagonal zero-weighted)
NCORES = 8
NB = B // NCORES    # samples per core
G = H // 128        # output-feature tiles (2)
K = H // 128        # contraction tiles (2)
CH = 512            # e-grid chunk = one PSUM bank
NCH = 5
EGP = CH * NCH      # 2560, padded grid (pad cols have ew == 0)

f32 = mybir.dt.float32
bf16 = mybir.dt.bfloat16
AF = mybir.ActivationFunctionType
ALU = mybir.AluOpType

_prog_cache = {}


def _build_program():
    nc = bass.Bass()

    def din(name, shape, dt):
        return nc.dram_tensor(name, list(shape), dt, kind="ExternalInput")

    hbf = din("hbf", (128, K, NB, V), bf16)
    hf32v = din("hf32", (128, K, NB, V), f32)
    xbf = din("xbf", (D, NB, V), bf16)
    xf32v = din("xf32", (D, NB, V), f32)
    ew = din("ew", (NB, T * EGP), bf16)
    w1 = din("w1", (128, K, T, 2, G, 128), bf16)
    rs = din("rs", (128, EGP), bf16)
    w2 = din("w2", (128, K, T, G, 128), bf16)
    wh = din("wh", (128, K, 3, G, 128), bf16)
    wi = din("wi", (D, 3, G, 128), bf16)
    wo12 = din("wo12", (128, K, 2, G, 128), bf16)
    wo3 = din("wo3", (128, K, D), bf16)
    b1d = din("bias1", (128, T, G), f32)
    b2d = din("bias2", (128, T, G), f32)
    bgd = din("biasg", (128, 3, G), f32)
    bod = din("biaso", (128, 2, G), f32)
    bo3d = din("bo3", (D, 1), f32)

    hnT = nc.dram_tensor("hnT", [128, K, NB, V], f32, kind="ExternalOutput")
    predT = nc.dram_tensor("predT", [D, NB, V], f32, kind="ExternalOutput")

    with tile.TileContext(nc) as tc:
        with (
            tc.tile_pool(name="singles", bufs=1) as singles,
            tc.tile_pool(name="work", bufs=2) as work,
            tc.tile_pool(name="m2pool", bufs=3) as m2pool,
            tc.tile_pool(name="psA", bufs=2, space="PSUM") as psA,
            tc.tile_pool(name="psB", bufs=1, space="PSUM") as psB,
        ):
            # ---------------- constants into SBUF ----------------
            # loads ordered by first use; late-needed ones go on the SWDGE
            # queue so the HWDGE queue isn't serialized in front of them
            def load(name, dram, shape, dt, eng=None):
                t = singles.tile(list(shape), dt, tag=name)
                (eng or nc.sync).dma_start(out=t, in_=dram[...])
                return t

            hbf_s = load("hbf", hbf, (128, K, NB, V), bf16)
            w1_s = singles.tile([128, K, T, 2, G, 128], bf16, tag="w1")
            nc.scalar.dma_start(out=w1_s[:, 0], in_=w1[:, 0])
            nc.sync.dma_start(out=w1_s[:, 1], in_=w1[:, 1])
            rs_s = load("rs", rs, (128, EGP), bf16)
            b1_s = load("b1", b1d, (128, T, G), f32)
            w2_s = load("w2", w2, (128, K, T, G, 128), bf16, nc.scalar)
            b2_s = load("b2", b2d, (128, T, G), f32)
            hf32_s = load("hf32", hf32v, (128, K, NB, V), f32, nc.gpsimd)
            xbf_s = load("xbf", xbf, (D, NB, V), bf16, nc.gpsimd)
            xf32_s = load("xf32", xf32v, (D, NB, V), f32, nc.gpsimd)
            wh_s = load("wh", wh, (128, K, 3, G, 128), bf16, nc.gpsimd)
            wi_s = load("wi", wi, (D, 3, G, 128), bf16, nc.gpsimd)
            wo12_s = load("wo12", wo12, (128, K, 2, G, 128), bf16, nc.gpsimd)
            wo3_s = load("wo3", wo3, (128, K, D), bf16, nc.gpsimd)
            bg_s = load("bg", bgd, (128, 3, G), f32, nc.gpsimd)
            bo_s = load("bo", bod, (128, 2, G), f32, nc.gpsimd)
            bo3_s = load("bo3", bo3d, (D, 1), f32, nc.gpsimd)

            agg_all = singles.tile([128, G, NB, V], f32, tag="agg")
            agg_bf = singles.tile([128, G, NB, V], bf16, tag="aggbf")
            hn_f = singles.tile([128, G, NB, V], f32, tag="hnf")
            hn_bf = singles.tile([128, G, NB, V], bf16, tag="hnbf")
            r_sb = singles.tile([128, G, NB, V], f32, tag="r_sb")
            i_sb = singles.tile([128, G, NB, V], f32, tag="i_sb")
            tmp = singles.tile([128, G, NB, V], f32, tag="tmp")
            n_sb = singles.tile([128, G, NB, V], f32, tag="n_sb")
            p1_bf = singles.tile([128, G, NB, V], bf16, tag="p1bf")
            p2_bf = singles.tile([128, G, NB, V], bf16, tag="p2bf")
            pred_sb = singles.tile([D, NB, V], f32, tag="pred")

            # ---------------- A/B node-level products (all samples) --------
            # A_vg[v,g] (rows 0-49) and B_vg[v,g] (rows 64-113) of the
            # selector-matmul stationary operand; evacuated via ACT while
            # the big constants are still streaming in.
            ab_sb = singles.tile([128, NB, T, H], bf16, tag="ab")
            nc.gpsimd.memset(
                ab_sb[32:64].rearrange("p b t g -> p (b t g)"), 0.0
            )
            nc.gpsimd.memset(
                ab_sb[96:128].rearrange("p b t g -> p (b t g)"), 0.0
            )
            for b in range(NB):
                pab = psA.tile([128, 3, CH], f32, tag="psa")
                for ab in range(2):
                    row0 = 0 if ab == 0 else 64
                    for t in range(T):
                        for k in range(K):
                            nc.tensor.matmul(
                                pab[row0 : row0 + V, t, 0:H],
                                hbf_s[:, k, b, :],
                                w1_s[:, k, t, ab].rearrange("p g m -> p (g m)"),
                                start=(k == 0),
                                stop=(k == K - 1),
                            )
                nc.vector.tensor_copy(
                    ab_sb[0:V, b].rearrange("p t g -> p (t g)"),
                    pab[0:V, 0:T, 0:H],
                )
                nc.vector.tensor_copy(
                    ab_sb[64 : 64 + V, b].rearrange("p t g -> p (t g)"),
                    pab[64 : 64 + V, 0:T, 0:H],
                )

            def emit_gru_mlp(b0):
                """GRU + output MLP for samples [b0, b0+1] (columns 2*V)."""
                bs = slice(b0, b0 + 2)
                NV2 = 2 * V
                nc.vector.tensor_copy(
                    agg_bf[:, :, bs, :], agg_all[:, :, bs, :]
                )
                xb = xbf_s[:, bs, :].rearrange("d b v -> d (b v)")
                for go in range(G):
                    pg = psB.tile([128, 2, CH], f32, tag="psb")
                    for gate in range(3):  # r, i, hh
                        for k in range(K):
                            nc.tensor.matmul(
                                pg[:, gate // 2, (gate % 2) * 128 : (gate % 2) * 128 + NV2],
                                wh_s[:, k, gate, go, :],
                                agg_bf[:, k, bs, :].rearrange("p b v -> p (b v)"),
                                start=(k == 0),
                                stop=(k == K - 1 and gate == 2),
                            )
                        if gate < 2:
                            nc.tensor.matmul(
                                pg[:, gate // 2, (gate % 2) * 128 : (gate % 2) * 128 + NV2],
                                wi_s[:, gate, go, :],
                                xb,
                                start=False,
                                stop=True,
                            )
                    # xn = Win @ inputs (chunk 1, cols 256:356)
                    nc.tensor.matmul(
                        pg[:, 1, 256 : 256 + NV2], wi_s[:, 2, go, :], xb,
                        start=True, stop=True,
                    )
                    rgo = r_sb[:, go, bs, :].rearrange("p b v -> p (b v)")
                    igo = i_sb[:, go, bs, :].rearrange("p b v -> p (b v)")
                    tgo = tmp[:, go, bs, :].rearrange("p b v -> p (b v)")
                    ngo = n_sb[:, go, bs, :].rearrange("p b v -> p (b v)")
                    nc.scalar.activation(
                        rgo, pg[:, 0, 0:NV2], AF.Sigmoid,
                        bias=bg_s[:, 0, go : go + 1],
                    )
                    nc.scalar.activation(
                        igo, pg[:, 0, 128 : 128 + NV2], AF.Sigmoid,
                        bias=bg_s[:, 1, go : go + 1],
                    )
                    nc.vector.tensor_mul(tgo, rgo, pg[:, 1, 0:NV2])
                    nc.vector.tensor_add(tgo, tgo, pg[:, 1, 256 : 256 + NV2])
                    nc.scalar.activation(
                        ngo, tgo, AF.Tanh, bias=bg_s[:, 2, go : go + 1]
                    )
                    # hidden_new = n + i*(hidden - n)
                    hgo = hf32_s[:, go, bs, :].rearrange("p b v -> p (b v)")
                    nc.vector.tensor_sub(tgo, hgo, ngo)
                    nc.vector.tensor_mul(tgo, igo, tgo)
                    nc.vector.tensor_add(
                        hn_f[:, go, bs, :].rearrange("p b v -> p (b v)"), ngo, tgo
                    )
                    nc.sync.dma_start(
                        out=hnT[:, go, bs], in_=hn_f[:, go, bs, :]
                    )
                    nc.vector.tensor_copy(hn_bf[:, go, bs, :], hn_f[:, go, bs, :])
                pm1 = psB.tile([128, 2, CH], f32, tag="psb")
                pm2 = psB.tile([128, 2, CH], f32, tag="psb")
                for go in range(G):
                    for k in range(K):
                        nc.tensor.matmul(
                            pm1[:, 0, go * 128 : go * 128 + NV2],
                            wo12_s[:, k, 0, go, :],
                            hn_bf[:, k, bs, :].rearrange("p b v -> p (b v)"),
                            start=(k == 0),
                            stop=(k == K - 1),
                        )
                    nc.vector.tensor_scalar(
                        p1_bf[:, go, bs, :].rearrange("p b v -> p (b v)"),
                        pm1[:, 0, go * 128 : go * 128 + NV2],
                        bo_s[:, 0, go : go + 1], 0.0,
                        op0=ALU.add, op1=ALU.max,
                    )
                for go in range(G):
                    for k in range(K):
                        nc.tensor.matmul(
                            pm2[:, 0, go * 128 : go * 128 + NV2],
                            wo12_s[:, k, 1, go, :],
                            p1_bf[:, k, bs, :].rearrange("p b v -> p (b v)"),
                            start=(k == 0),
                            stop=(k == K - 1),
                        )
                    nc.vector.tensor_scalar(
                        p2_bf[:, go, bs, :].rearrange("p b v -> p (b v)"),
                        pm2[:, 0, go * 128 : go * 128 + NV2],
                        bo_s[:, 1, go : go + 1], 0.0,
                        op0=ALU.add, op1=ALU.max,
                    )
                for k in range(K):
                    nc.tensor.matmul(
                        pm2[0:D, 1, 0:NV2],
                        wo3_s[:, k, :],
                        p2_bf[:, k, bs, :].rearrange("p b v -> p (b v)"),
                        start=(k == 0),
                        stop=(k == K - 1),
                    )
                nc.vector.scalar_tensor_tensor(
                    pred_sb[:, bs, :].rearrange("d b v -> d (b v)"),
                    pm2[0:D, 1, 0:NV2],
                    bo3_s[0:D, 0:1],
                    xf32_s[:, bs, :].rearrange("d b v -> d (b v)"),
                    op0=ALU.add,
                    op1=ALU.add,
                )
                nc.sync.dma_start(out=predT[:, bs], in_=pred_sb[:, bs, :])

            # ---------------- per-sample edge pipeline ----------------
            ew_tiles = {}
            for b in range(NB):
                ewt = work.tile([128, T, EGP], bf16, tag="ew")
                nc.gpsimd.dma_start(
                    out=ewt.rearrange("p t e -> p (t e)"),
                    in_=ew[b : b + 1, :].broadcast_to([128, T * EGP]),
                )
                ew_tiles[b] = ewt
                # m1[g, e] = tanh(A_vg[r(e), g] + B_vg[s(e), g] + b1[g]) via
                # a single-K-pass selector matmul (RS rows 50-63/114-127
                # are zero, killing the garbage lhsT rows).
                m1f = work.tile([128, K, T, EGP], bf16, tag="m1")
                for kh in range(K):
                    for t in range(T):
                        lhsT = ab_sb[:, b, t, kh * 128 : (kh + 1) * 128]
                        bias = b1_s[:, t, kh : kh + 1]
                        pa = psA.tile([128, 3, CH], f32, tag="psa")
                        for c in range(3):
                            nc.tensor.matmul(
                                pa[:, c, :],
                                lhsT,
                                rs_s[:, c * CH : (c + 1) * CH],
                                start=True,
                                stop=True,
                            )
                        nc.scalar.activation(
                            m1f[:, kh, t, 0 : 3 * CH],
                            pa[:, 0:3, :],
                            AF.Tanh,
                            bias=bias,
                        )
                        pb = psB.tile([128, 2, CH], f32, tag="psb")
                        for c in range(2):
                            nc.tensor.matmul(
                                pb[:, c, :],
                                lhsT,
                                rs_s[:, (3 + c) * CH : (4 + c) * CH],
                                start=True,
                                stop=True,
                            )
                        nc.scalar.activation(
                            m1f[:, kh, t, 3 * CH : EGP],
                            pb[:, 0:2, :],
                            AF.Tanh,
                            bias=bias,
                        )
                # m2 = tanh(m1 @ W2[t]^T + b2)
                for go in range(G):
                    m2 = m2pool.tile([128, T, EGP], bf16, tag="m2")
                    for t in range(T):
                        bias = b2_s[:, t, go : go + 1]
                        pa = psA.tile([128, 3, CH], f32, tag="psa")
                        for k in range(K):
                            for c in range(3):
                                nc.tensor.matmul(
                                    pa[:, c, :],
                                    w2_s[:, k, t, go, :],
                                    m1f[:, k, t, c * CH : (c + 1) * CH],
                                    start=(k == 0),
                                    stop=(k == K - 1),
                                )
                        nc.scalar.activation(
                            m2[:, t, 0 : 3 * CH], pa[:, 0:3, :], AF.Tanh, bias=bias
                        )
                        pb = psB.tile([128, 2, CH], f32, tag="psb")
                        for k in range(K):
                            for c in range(2):
                                nc.tensor.matmul(
                                    pb[:, c, :],
                                    w2_s[:, k, t, go, :],
                                    m1f[:, k, t, (3 + c) * CH : (4 + c) * CH],
                                    start=(k == 0),
                                    stop=(k == K - 1),
                                )
                        nc.scalar.activation(
                            m2[:, t, 3 * CH : EGP], pb[:, 0:2, :], AF.Tanh, bias=bias
                        )
                        # weight by edges as each type lands (keeps the DVE
                        # chain interleaved under the ACT evacuations)
                        nc.vector.tensor_mul(m2[:, t], m2[:, t], ewt[:, t])
                        if t == 1:
                            ps2 = work.tile([128, EGP], bf16, tag="ps2")
                            nc.vector.tensor_add(ps2, m2[:, 0], m2[:, 1])
                    nc.vector.tensor_add(ps2, ps2, m2[:, 2])
                    # aggregate over senders: agg[r] = sum_s ps2[r*V+s]
                    nc.vector.reduce_sum(
                        agg_all[:, go, b, :],
                        ps2[:, 0:EG].rearrange("p (r s) -> p r s", s=V),
                        axis=mybir.AxisListType.X,
                    )
                if b == 1:
                    emit_gru_mlp(0)
            emit_gru_mlp(2)

    return nc


def _host_prep(inputs, hidden, edges, W1, b1, W2, b2, Whr, Whi, Whh,
               Wir, bir, Wii, bii, Win, bin_w, Wo1, bo1, Wo2, bo2, Wo3, bo3):
    """Build per-core input maps (all numpy, host-side layout only)."""
    asf = np.ascontiguousarray

    def b16(x):
        return asf(x.astype(ml_dtypes.bfloat16))

    # hidden^T: [p, k, b, v]
    hT = hidden.transpose(2, 0, 1).reshape(K, 128, B, V).transpose(1, 0, 2, 3)
    xT = inputs.transpose(2, 0, 1)  # [d, b, v]

    # edge-weight grid (recv-major), diag zero, scaled by 1/(T*(V-1)),
    # padded to EGP columns (pad cols zero)
    mask = np.ones((V, V)) - np.eye(V)
    send, recv = np.where(mask)
    M = np.zeros((B, V, V, T), np.float32)
    M[:, send, recv, :] = edges[:, :, 1:]
    ewg = M.transpose(0, 3, 2, 1).reshape(B, T, EG) / (T * (V - 1))
    ewp = np.zeros((B, T, EGP), np.float32)
    ewp[:, :, :EG] = ewg
    ewp = ewp.reshape(B, T * EGP)

    # weights as lhsT layouts
    W1a, W1b = W1[:, :, :H], W1[:, :, H:]

    def lhsT(wmat):  # [out, in] -> [p, k, g, m] with in = k*128+p, out = g*128+m
        return wmat.T.reshape(K, 128, G, 128).transpose(1, 0, 2, 3)

    w1_h = np.stack(
        [
            np.stack([lhsT(W1a[t]), lhsT(W1b[t])], axis=2)  # [p,k,ab,g,m]
            for t in range(T)
        ],
        axis=2,
    )  # [p, k, t, ab, g, m]
    w2_h = np.stack([lhsT(W2[t]) for t in range(T)], axis=2)  # [p,k,t,g,m]
    wh_h = np.stack([lhsT(Whr), lhsT(Whi), lhsT(Whh)], axis=2)
    wi_h = np.stack(
        [Wir.T.reshape(D, G, 128), Wii.T.reshape(D, G, 128),
         Win.T.reshape(D, G, 128)],
        axis=1,
    )  # [d, gate, g, m]
    wo12_h = np.stack([lhsT(Wo1), lhsT(Wo2)], axis=2)  # [p,k,o,g,m]
    wo3_h = Wo3.T.reshape(K, 128, D).transpose(1, 0, 2)  # [p,k,d]

    def pcol(vec):  # [H] -> [p, g]
        return vec.reshape(G, 128).T

    b1_h = np.stack([pcol(b1[t]) for t in range(T)], axis=1)  # [p,t,g]
    b2_h = np.stack([pcol(b2[t]) for t in range(T)], axis=1)
    bg_h = np.stack([pcol(bir), pcol(bii), pcol(bin_w)], axis=1)
    bo_h = np.stack([pcol(bo1), pcol(bo2)], axis=1)
    bo3_h = bo3.reshape(D, 1).astype(np.float32)

    # selector for the m1pre matmul: rows 0-49 pick recv (e // V), rows
    # 64-113 pick send (e % V); all other rows and pad columns zero.
    rs_h = np.zeros((128, EGP), np.float32)
    e = np.arange(EG)
    rs_h[e // V, e] = 1.0
    rs_h[64 + e % V, e] = 1.0

    shared = {
        "rs": b16(rs_h),
        "w1": b16(w1_h), "w2": b16(w2_h), "wh": b16(wh_h), "wi": b16(wi_h),
        "wo12": b16(wo12_h), "wo3": b16(wo3_h),
        "bias1": asf(b1_h.astype(np.float32)),
        "bias2": asf(b2_h.astype(np.float32)),
        "biasg": asf(bg_h.astype(np.float32)),
        "biaso": asf(bo_h.astype(np.float32)),
        "bo3": bo3_h,
    }
    in_maps = []
    for c in range(NCORES):
        sl = slice(c * NB, (c + 1) * NB)
        m = dict(shared)
        m["hbf"] = b16(hT[:, :, sl])
        m["hf32"] = asf(hT[:, :, sl].astype(np.float32))
        m["xbf"] = b16(xT[:, sl])
        m["xf32"] = asf(xT[:, sl].astype(np.float32))
        m["ew"] = b16(ewp[sl])
        in_maps.append(m)
    return in_maps


def kernel(**inputs):
    if "prog" not in _prog_cache:
        _prog_cache["prog"] = _build_program()
    nc = _prog_cache["prog"]

    in_maps = _host_prep(**{k: np.asarray(v) for k, v in inputs.items()})
    res = run_bass_kernel_spmd(nc, in_maps, list(range(NCORES)))

    pred = np.empty((B, V, D), np.float32)
    hidden_new = np.empty((B, V, H), np.float32)
    for c in range(NCORES):
        sl = slice(c * NB, (c + 1) * NB)
        hnT = np.asarray(res.results[c]["hnT"], np.float32)  # [p,k,b,v]
        predT = np.asarray(res.results[c]["predT"], np.float32)  # [d,b,v]
        hidden_new[sl] = hnT.transpose(2, 3, 1, 0).reshape(NB, V, H)
        pred[sl] = predT.transpose(1, 2, 0)
    return pred, hidden_new


# revision 41
# speedup vs baseline: 1.1224x; 1.1224x over previous
"""DNRI message-passing step on 8 Trainium2 NeuronCores.

Strategy: data-parallel over batch B=32 -> 4 samples per core; params
replicated. Per sample, the per-edge first-layer matmul is factorized into
per-node products A_vg = hidden @ W1a^T, B_vg = hidden @ W1b^T, and the
dense 50x50 (recv-major) edge grid is produced by a single-K-pass
"selector" matmul whose stationary operand stacks [A_vg; B_vg] and whose
moving operand is a constant 0/1 recv/send selector — the PE does the
edge-grid gather/expansion for free. Only the second edge-MLP layer runs
at edge granularity. Edge aggregation is a contiguous strided reduction
over the send axis (no gather/scatter). All heavy matmuls in bf16 with
fp32 PSUM accumulation; GRU/output MLP batched over the 4 samples.

The edge grid is padded 2500 -> 2560 so every PSUM chunk is a full
512-col bank; PSUM groups are a 4-bank tile + a 1-bank tile from one
bufs=2 pool, so matmuls of group i+1 overlap ACT evacuation of group i.
"""

import numpy as np
import ml_dtypes

import concourse.bass as bass
import concourse.mybir as mybir
import concourse.tile as tile
from concourse.bass_utils import run_bass_kernel_spmd

# ----------------------------------------------------------------------------
# Workaround for walrus "Too many sync wait commands": the neuronxcc in this
# container accepts at most one semaphore wait per instruction, but Tile's
# sem-assignment can emit several. After scheduling, hoist excess waits onto
# same-engine InstNoOp carriers inserted immediately before the instruction.
# ----------------------------------------------------------------------------
_MAXW = 1
_orig_tile_exit = tile.TileContext.__exit__
_uid = [0]


def _split_waits(nc):
    for func in nc.m.functions:
        for bb in func.blocks:
            new = []
            for inst in bb.instructions:
                si = inst.sync_info
                waits = list(si.on_wait) if si and si.on_wait else []
                if len(waits) > _MAXW:
                    n_keep = len(waits) % _MAXW or _MAXW
                    covered = len(waits) - n_keep
                    for j in range(0, covered, _MAXW):
                        _uid[0] += 1
                        carrier = mybir.InstNoOp(
                            name=f"waitsplit-{_uid[0]}",
                            sync_info=mybir.SyncInfo(
                                on_wait=waits[j : j + _MAXW], on_update=[]
                            ),
                            bass_nofuse=True,
                            engine=inst.engine,
                        )
                        nc.register_instruction(carrier, overwrite=True)
                        new.append(carrier)
                    si.on_wait = waits[covered:]
                new.append(inst)
            bb.instructions[:] = new


def _patched_tile_exit(self, exc_type, exc_val, exc_tb):
    r = _orig_tile_exit(self, exc_type, exc_val, exc_tb)
    if exc_type is None:
        _split_waits(self.nc)
    return r


if getattr(tile.TileContext.__exit__, "__name__", "") != "_patched_tile_exit":
    tile.TileContext.__exit__ = _patched_tile_exit

# ---------------------------------------------------------------------------
# Problem constants (hardcoded per spec)
# ---------------------------------------------------------------------------
B, V, D, H, ET = 32, 50, 6, 256, 4
T = ET - 1          # non-skipped edge types
E = V * (V - 1)     # 2450 real edges
EG = V * V          # 2500 grid edges (diagonal zero-weighted)
NCORES = 8
NB = B // NCORES    # samples per core
G = H // 128        # output-feature tiles (2)
K = H // 128        # contraction tiles (2)
CH = 512            # e-grid chunk = one PSUM bank
NCH = 5
EGP = CH * NCH      # 2560, padded grid (pad cols have ew == 0)

f32 = mybir.dt.float32
bf16 = mybir.dt.bfloat16
AF = mybir.ActivationFunctionType
ALU = mybir.AluOpType

_prog_cache = {}


def _build_program():
    nc = bass.Bass()

    def din(name, shape, dt):
        return nc.dram_tensor(name, list(shape), dt, kind="ExternalInput")

    hbf = din("hbf", (128, K, NB, V), bf16)
    hf32v = din("hf32", (128, K, NB, V), f32)
    xbf = din("xbf", (D, NB, V), bf16)
    xf32v = din("xf32", (D, NB, V), f32)
    ew = din("ew", (NB, T * EGP), bf16)
    w1 = din("w1", (128, K, T, 2, G, 128), bf16)
    rs = din("rs", (128, EGP), bf16)
    w2 = din("w2", (128, K, T, G, 128), bf16)
    wh = din("wh", (128, K, 3, G, 128), bf16)
    wi = din("wi", (D, 3, G, 128), bf16)
    wo12 = din("wo12", (128, K, 2, G, 128), bf16)
    wo3 = din("wo3", (128, K, D), bf16)
    b1d = din("bias1", (128, T, G), f32)
    b2d = din("bias2", (128, T, G), f32)
    bgd = din("biasg", (128, 3, G), f32)
    bod = din("biaso", (128, 2, G), f32)
    bo3d = din("bo3", (D, 1), f32)

    hnT = nc.dram_tensor("hnT", [128, K, NB, V], f32, kind="ExternalOutput")
    predT = nc.dram_tensor("predT", [D, NB, V], f32, kind="ExternalOutput")

    with tile.TileContext(nc) as tc:
        with (
            tc.tile_pool(name="singles", bufs=1) as singles,
            tc.tile_pool(name="work", bufs=2) as work,
            tc.tile_pool(name="m2pool", bufs=3) as m2pool,
            tc.tile_pool(name="psA", bufs=2, space="PSUM") as psA,
            tc.tile_pool(name="psB", bufs=1, space="PSUM") as psB,
        ):
            # ---------------- constants into SBUF ----------------
            # loads ordered by first use; late-needed ones go on the SWDGE
            # queue so the HWDGE queue isn't serialized in front of them
            def load(name, dram, shape, dt, eng=None):
                t = singles.tile(list(shape), dt, tag=name)
                (eng or nc.sync).dma_start(out=t, in_=dram[...])
                return t

            hbf_s = load("hbf", hbf, (128, K, NB, V), bf16)
            w1_s = singles.tile([128, K, T, 2, G, 128], bf16, tag="w1")
            nc.scalar.dma_start(out=w1_s[:, 0], in_=w1[:, 0])
            nc.sync.dma_start(out=w1_s[:, 1], in_=w1[:, 1])
            rs_s = load("rs", rs, (128, EGP), bf16)
            b1_s = load("b1", b1d, (128, T, G), f32)
            w2_s = load("w2", w2, (128, K, T, G, 128), bf16, nc.scalar)
            b2_s = load("b2", b2d, (128, T, G), f32)
            hf32_s = load("hf32", hf32v, (128, K, NB, V), f32, nc.gpsimd)
            xbf_s = load("xbf", xbf, (D, NB, V), bf16, nc.gpsimd)
            xf32_s = load("xf32", xf32v, (D, NB, V), f32, nc.gpsimd)
            wh_s = load("wh", wh, (128, K, 3, G, 128), bf16, nc.gpsimd)
            wi_s = load("wi", wi, (D, 3, G, 128), bf16, nc.gpsimd)
            wo12_s = load("wo12", wo12, (128, K, 2, G, 128), bf16, nc.gpsimd)
            wo3_s = load("wo3", wo3, (128, K, D), bf16, nc.gpsimd)
            bg_s = load("bg", bgd, (128, 3, G), f32, nc.gpsimd)
            bo_s = load("bo", bod, (128, 2, G), f32, nc.gpsimd)
            bo3_s = load("bo3", bo3d, (D, 1), f32, nc.gpsimd)

            agg_all = singles.tile([128, G, NB, V], f32, tag="agg")
            agg_bf = singles.tile([128, G, NB, V], bf16, tag="aggbf")
            hn_f = singles.tile([128, G, NB, V], f32, tag="hnf")
            hn_bf = singles.tile([128, G, NB, V], bf16, tag="hnbf")
            r_sb = singles.tile([128, G, NB, V], f32, tag="r_sb")
            i_sb = singles.tile([128, G, NB, V], f32, tag="i_sb")
            tmp = singles.tile([128, G, NB, V], f32, tag="tmp")
            n_sb = singles.tile([128, G, NB, V], f32, tag="n_sb")
            p1_bf = singles.tile([128, G, NB, V], bf16, tag="p1bf")
            p2_bf = singles.tile([128, G, NB, V], bf16, tag="p2bf")
            pred_sb = singles.tile([D, NB, V], f32, tag="pred")

            # ---------------- A/B node-level products (all samples) --------
            # A_vg[v,g] (rows 0-49) and B_vg[v,g] (rows 64-113) of the
            # selector-matmul stationary operand; evacuated via ACT while
            # the big constants are still streaming in.
            ab_sb = singles.tile([128, NB, T, H], bf16, tag="ab")
            nc.gpsimd.memset(
                ab_sb[32:64].rearrange("p b t g -> p (b t g)"), 0.0
            )
            nc.gpsimd.memset(
                ab_sb[96:128].rearrange("p b t g -> p (b t g)"), 0.0
            )
            for b in range(NB):
                pab = psA.tile([128, 3, CH], f32, tag="psa")
                for ab in range(2):
                    row0 = 0 if ab == 0 else 64
                    for t in range(T):
                        for k in range(K):
                            nc.tensor.matmul(
                                pab[row0 : row0 + V, t, 0:H],
                                hbf_s[:, k, b, :],
                                w1_s[:, k, t, ab].rearrange("p g m -> p (g m)"),
                                start=(k == 0),
                                stop=(k == K - 1),
                            )
                nc.vector.tensor_copy(
                    ab_sb[0:V, b].rearrange("p t g -> p (t g)"),
                    pab[0:V, 0:T, 0:H],
                )
                nc.vector.tensor_copy(
                    ab_sb[64 : 64 + V, b].rearrange("p t g -> p (t g)"),
                    pab[64 : 64 + V, 0:T, 0:H],
                )

            def emit_gru_mlp(b0):
                """GRU + output MLP for samples [b0, b0+1] (columns 2*V)."""
                bs = slice(b0, b0 + 2)
                NV2 = 2 * V
                nc.vector.tensor_copy(
                    agg_bf[:, :, bs, :], agg_all[:, :, bs, :]
                )
                xb = xbf_s[:, bs, :].rearrange("d b v -> d (b v)")
                for go in range(G):
                    pg = psB.tile([128, 2, CH], f32, tag="psb")
                    for gate in range(3):  # r, i, hh
                        for k in range(K):
                            nc.tensor.matmul(
                                pg[:, gate // 2, (gate % 2) * 128 : (gate % 2) * 128 + NV2],
                                wh_s[:, k, gate, go, :],
                                agg_bf[:, k, bs, :].rearrange("p b v -> p (b v)"),
                                start=(k == 0),
                                stop=(k == K - 1 and gate == 2),
                            )
                        if gate < 2:
                            nc.tensor.matmul(
                                pg[:, gate // 2, (gate % 2) * 128 : (gate % 2) * 128 + NV2],
                                wi_s[:, gate, go, :],
                                xb,
                                start=False,
                                stop=True,
                            )
                    # xn = Win @ inputs (chunk 1, cols 256:356)
                    nc.tensor.matmul(
                        pg[:, 1, 256 : 256 + NV2], wi_s[:, 2, go, :], xb,
                        start=True, stop=True,
                    )
                    rgo = r_sb[:, go, bs, :].rearrange("p b v -> p (b v)")
                    igo = i_sb[:, go, bs, :].rearrange("p b v -> p (b v)")
                    tgo = tmp[:, go, bs, :].rearrange("p b v -> p (b v)")
                    ngo = n_sb[:, go, bs, :].rearrange("p b v -> p (b v)")
                    nc.scalar.activation(
                        rgo, pg[:, 0, 0:NV2], AF.Sigmoid,
                        bias=bg_s[:, 0, go : go + 1],
                    )
                    nc.scalar.activation(
                        igo, pg[:, 0, 128 : 128 + NV2], AF.Sigmoid,
                        bias=bg_s[:, 1, go : go + 1],
                    )
                    nc.vector.tensor_mul(tgo, rgo, pg[:, 1, 0:NV2])
                    nc.vector.tensor_add(tgo, tgo, pg[:, 1, 256 : 256 + NV2])
                    nc.scalar.activation(
                        ngo, tgo, AF.Tanh, bias=bg_s[:, 2, go : go + 1]
                    )
                    # hidden_new = n + i*(hidden - n)
                    hgo = hf32_s[:, go, bs, :].rearrange("p b v -> p (b v)")
                    nc.vector.tensor_sub(tgo, hgo, ngo)
                    nc.vector.tensor_mul(tgo, igo, tgo)
                    nc.vector.tensor_add(
                        hn_f[:, go, bs, :].rearrange("p b v -> p (b v)"), ngo, tgo
                    )
                    nc.sync.dma_start(
                        out=hnT[:, go, bs], in_=hn_f[:, go, bs, :]
                    )
                    nc.vector.tensor_copy(hn_bf[:, go, bs, :], hn_f[:, go, bs, :])
                pm1 = psB.tile([128, 2, CH], f32, tag="psb")
                pm2 = psB.tile([128, 2, CH], f32, tag="psb")
                for go in range(G):
                    for k in range(K):
                        nc.tensor.matmul(
                            pm1[:, 0, go * 128 : go * 128 + NV2],
                            wo12_s[:, k, 0, go, :],
                            hn_bf[:, k, bs, :].rearrange("p b v -> p (b v)"),
                            start=(k == 0),
                            stop=(k == K - 1),
                        )
                    nc.vector.tensor_scalar(
                        p1_bf[:, go, bs, :].rearrange("p b v -> p (b v)"),
                        pm1[:, 0, go * 128 : go * 128 + NV2],
                        bo_s[:, 0, go : go + 1], 0.0,
                        op0=ALU.add, op1=ALU.max,
                    )
                for go in range(G):
                    for k in range(K):
                        nc.tensor.matmul(
                            pm2[:, 0, go * 128 : go * 128 + NV2],
                            wo12_s[:, k, 1, go, :],
                            p1_bf[:, k, bs, :].rearrange("p b v -> p (b v)"),
                            start=(k == 0),
                            stop=(k == K - 1),
                        )
                    nc.vector.tensor_scalar(
                        p2_bf[:, go, bs, :].rearrange("p b v -> p (b v)"),
                        pm2[:, 0, go * 128 : go * 128 + NV2],
                        bo_s[:, 1, go : go + 1], 0.0,
                        op0=ALU.add, op1=ALU.max,
                    )
                for k in range(K):
                    nc.tensor.matmul(
                        pm2[0:D, 1, 0:NV2],
                        wo3_s[:, k, :],
                        p2_bf[:, k, bs, :].rearrange("p b v -> p (b v)"),
                        start=(k == 0),
                        stop=(k == K - 1),
                    )
                nc.vector.scalar_tensor_tensor(
                    pred_sb[:, bs, :].rearrange("d b v -> d (b v)"),
                    pm2[0:D, 1, 0:NV2],
                    bo3_s[0:D, 0:1],
                    xf32_s[:, bs, :].rearrange("d b v -> d (b v)"),
                    op0=ALU.add,
                    op1=ALU.add,
                )
                nc.sync.dma_start(out=predT[:, bs], in_=pred_sb[:, bs, :])

            # ---------------- per-sample edge pipeline ----------------
            ew_tiles = {}
            for b in range(NB):
                ewt = work.tile([128, T, EGP], bf16, tag="ew")
                nc.gpsimd.dma_start(
                    out=ewt.rearrange("p t e -> p (t e)"),
                    in_=ew[b : b + 1, :].broadcast_to([128, T * EGP]),
                )
                ew_tiles[b] = ewt
                # m1[g, e] = tanh(A_vg[r(e), g] + B_vg[s(e), g] + b1[g]) via
                # a single-K-pass selector matmul (RS rows 50-63/114-127
                # are zero, killing the garbage lhsT rows).
                m1f = work.tile([128, K, T, EGP], bf16, tag="m1")
                for kh in range(K):
                    for t in range(T):
                        lhsT = ab_sb[:, b, t, kh * 128 : (kh + 1) * 128]
                        bias = b1_s[:, t, kh : kh + 1]
                        pa = psA.tile([128, 3, CH], f32, tag="psa")
                        for c in range(3):
                            nc.tensor.matmul(
                                pa[:, c, :],
                                lhsT,
                                rs_s[:, c * CH : (c + 1) * CH],
                                start=True,
                                stop=True,
                            )
                        nc.scalar.activation(
                            m1f[:, kh, t, 0 : 3 * CH],
                            pa[:, 0:3, :],
                            AF.Tanh,
                            bias=bias,
                        )
                        pb = psB.tile([128, 2, CH], f32, tag="psb")
                        for c in range(2):
                            nc.tensor.matmul(
                                pb[:, c, :],
                                lhsT,
                                rs_s[:, (3 + c) * CH : (4 + c) * CH],
                                start=True,
                                stop=True,
                            )
                        nc.scalar.activation(
                            m1f[:, kh, t, 3 * CH : EGP],
                            pb[:, 0:2, :],
                            AF.Tanh,
                            bias=bias,
                        )
                # m2 = tanh(m1 @ W2[t]^T + b2)
                for go in range(G):
                    m2 = m2pool.tile([128, T, EGP], bf16, tag="m2")
                    for t in range(T):
                        bias = b2_s[:, t, go : go + 1]
                        pa = psA.tile([128, 3, CH], f32, tag="psa")
                        for k in range(K):
                            for c in range(3):
                                nc.tensor.matmul(
                                    pa[:, c, :],
                                    w2_s[:, k, t, go, :],
                                    m1f[:, k, t, c * CH : (c + 1) * CH],
                                    start=(k == 0),
                                    stop=(k == K - 1),
                                )
                        nc.scalar.activation(
                            m2[:, t, 0 : 3 * CH], pa[:, 0:3, :], AF.Tanh, bias=bias
                        )
                        pb = psB.tile([128, 2, CH], f32, tag="psb")
                        for k in range(K):
                            for c in range(2):
                                nc.tensor.matmul(
                                    pb[:, c, :],
                                    w2_s[:, k, t, go, :],
                                    m1f[:, k, t, (3 + c) * CH : (4 + c) * CH],
                                    start=(k == 0),
                                    stop=(k == K - 1),
                                )
                        nc.scalar.activation(
                            m2[:, t, 3 * CH : EGP], pb[:, 0:2, :], AF.Tanh, bias=bias
                        )
                        # weight by edges as each type lands (keeps the DVE
                        # chain interleaved under the ACT evacuations)
                        nc.vector.tensor_mul(m2[:, t], m2[:, t], ewt[:, t])
                        if t == 1:
                            ps2 = work.tile([128, EGP], bf16, tag="ps2")
                            nc.vector.tensor_add(ps2, m2[:, 0], m2[:, 1])
                    nc.vector.tensor_add(ps2, ps2, m2[:, 2])
                    # aggregate over senders: agg[r] = sum_s ps2[r*V+s]
                    nc.vector.reduce_sum(
                        agg_all[:, go, b, :],
                        ps2[:, 0:EG].rearrange("p (r s) -> p r s", s=V),
                        axis=mybir.AxisListType.X,
                    )
            emit_gru_mlp(0)
            emit_gru_mlp(2)

    return nc


def _host_prep(inputs, hidden, edges, W1, b1, W2, b2, Whr, Whi, Whh,
               Wir, bir, Wii, bii, Win, bin_w, Wo1, bo1, Wo2, bo2, Wo3, bo3):
    """Build per-core input maps (all numpy, host-side layout only)."""
    asf = np.ascontiguousarray

    def b16(x):
        return asf(x.astype(ml_dtypes.bfloat16))

    # hidden^T: [p, k, b, v]
    hT = hidden.transpose(2, 0, 1).reshape(K, 128, B, V).transpose(1, 0, 2, 3)
    xT = inputs.transpose(2, 0, 1)  # [d, b, v]

    # edge-weight grid (recv-major), diag zero, scaled by 1/(T*(V-1)),
    # padded to EGP columns (pad cols zero)
    mask = np.ones((V, V)) - np.eye(V)
    send, recv = np.where(mask)
    M = np.zeros((B, V, V, T), np.float32)
    M[:, send, recv, :] = edges[:, :, 1:]
    ewg = M.transpose(0, 3, 2, 1).reshape(B, T, EG) / (T * (V - 1))
    ewp = np.zeros((B, T, EGP), np.float32)
    ewp[:, :, :EG] = ewg
    ewp = ewp.reshape(B, T * EGP)

    # weights as lhsT layouts
    W1a, W1b = W1[:, :, :H], W1[:, :, H:]

    def lhsT(wmat):  # [out, in] -> [p, k, g, m] with in = k*128+p, out = g*128+m
        return wmat.T.reshape(K, 128, G, 128).transpose(1, 0, 2, 3)

    w1_h = np.stack(
        [
            np.stack([lhsT(W1a[t]), lhsT(W1b[t])], axis=2)  # [p,k,ab,g,m]
            for t in range(T)
        ],
        axis=2,
    )  # [p, k, t, ab, g, m]
    w2_h = np.stack([lhsT(W2[t]) for t in range(T)], axis=2)  # [p,k,t,g,m]
    wh_h = np.stack([lhsT(Whr), lhsT(Whi), lhsT(Whh)], axis=2)
    wi_h = np.stack(
        [Wir.T.reshape(D, G, 128), Wii.T.reshape(D, G, 128),
         Win.T.reshape(D, G, 128)],
        axis=1,
    )  # [d, gate, g, m]
    wo12_h = np.stack([lhsT(Wo1), lhsT(Wo2)], axis=2)  # [p,k,o,g,m]
    wo3_h = Wo3.T.reshape(K, 128, D).transpose(1, 0, 2)  # [p,k,d]

    def pcol(vec):  # [H] -> [p, g]
        return vec.reshape(G, 128).T

    b1_h = np.stack([pcol(b1[t]) for t in range(T)], axis=1)  # [p,t,g]
    b2_h = np.stack([pcol(b2[t]) for t in range(T)], axis=1)
    bg_h = np.stack([pcol(bir), pcol(bii), pcol(bin_w)], axis=1)
    bo_h = np.stack([pcol(bo1), pcol(bo2)], axis=1)
    bo3_h = bo3.reshape(D, 1).astype(np.float32)

    # selector for the m1pre matmul: rows 0-49 pick recv (e // V), rows
    # 64-113 pick send (e % V); all other rows and pad columns zero.
    rs_h = np.zeros((128, EGP), np.float32)
    e = np.arange(EG)
    rs_h[e // V, e] = 1.0
    rs_h[64 + e % V, e] = 1.0

    shared = {
        "rs": b16(rs_h),
        "w1": b16(w1_h), "w2": b16(w2_h), "wh": b16(wh_h), "wi": b16(wi_h),
        "wo12": b16(wo12_h), "wo3": b16(wo3_h),
        "bias1": asf(b1_h.astype(np.float32)),
        "bias2": asf(b2_h.astype(np.float32)),
        "biasg": asf(bg_h.astype(np.float32)),
        "biaso": asf(bo_h.astype(np.float32)),
        "bo3": bo3_h,
    }
    in_maps = []
    for c in range(NCORES):
        sl = slice(c * NB, (c + 1) * NB)
        m = dict(shared)
        m["hbf"] = b16(hT[:, :, sl])
        m["hf32"] = asf(hT[:, :, sl].astype(np.float32))
        m["xbf"] = b16(xT[:, sl])
        m["xf32"] = asf(xT[:, sl].astype(np.float32))
        m["ew"] = b16(ewp[sl])
        in_maps.append(m)
    return in_maps


def kernel(**inputs):
    if "prog" not in _prog_cache:
        _prog_cache["prog"] = _build_program()
    nc = _prog_cache["prog"]

    in_maps = _host_prep(**{k: np.asarray(v) for k, v in inputs.items()})
    res = run_bass_kernel_spmd(nc, in_maps, list(range(NCORES)))

    pred = np.empty((B, V, D), np.float32)
    hidden_new = np.empty((B, V, H), np.float32)
    for c in range(NCORES):
        sl = slice(c * NB, (c + 1) * NB)
        hnT = np.asarray(res.results[c]["hnT"], np.float32)  # [p,k,b,v]
        predT = np.asarray(res.results[c]["predT"], np.float32)  # [d,b,v]
        hidden_new[sl] = hnT.transpose(2, 3, 1, 0).reshape(NB, V, H)
        pred[sl] = predT.transpose(1, 2, 0)
    return pred, hidden_new


# revision 43
# speedup vs baseline: 1.1546x; 1.0287x over previous
"""DNRI message-passing step on 8 Trainium2 NeuronCores.

Strategy: data-parallel over batch B=32 -> 4 samples per core; params
replicated. Per sample, the per-edge first-layer matmul is factorized into
per-node products A_vg = hidden @ W1a^T, B_vg = hidden @ W1b^T, and the
dense 50x50 (recv-major) edge grid is produced by a single-K-pass
"selector" matmul whose stationary operand stacks [A_vg; B_vg] and whose
moving operand is a constant 0/1 recv/send selector — the PE does the
edge-grid gather/expansion for free. Only the second edge-MLP layer runs
at edge granularity. Edge aggregation is a contiguous strided reduction
over the send axis (no gather/scatter). All heavy matmuls in bf16 with
fp32 PSUM accumulation; GRU/output MLP batched over the 4 samples.

The edge grid is padded 2500 -> 2560 so every PSUM chunk is a full
512-col bank; each PSUM group is a 3-bank tile (double-buffered pool)
plus a 2-bank tile, so matmuls of group i+1 overlap the ACT tanh
evacuation of group i and the pipeline stays ACT-paced (~ACT busy 75%+
of kernel time). Measured ~176 us/core on TRN2 at rel err ~7e-4.
"""

import numpy as np
import ml_dtypes

import concourse.bass as bass
import concourse.mybir as mybir
import concourse.tile as tile
from concourse.bass_utils import run_bass_kernel_spmd

# ----------------------------------------------------------------------------
# Workaround for walrus "Too many sync wait commands": the neuronxcc in this
# container accepts at most one semaphore wait per instruction, but Tile's
# sem-assignment can emit several. After scheduling, hoist excess waits onto
# same-engine InstNoOp carriers inserted immediately before the instruction.
# ----------------------------------------------------------------------------
_MAXW = 1
_orig_tile_exit = tile.TileContext.__exit__
_uid = [0]


def _split_waits(nc):
    for func in nc.m.functions:
        for bb in func.blocks:
            new = []
            for inst in bb.instructions:
                si = inst.sync_info
                waits = list(si.on_wait) if si and si.on_wait else []
                if len(waits) > _MAXW:
                    n_keep = len(waits) % _MAXW or _MAXW
                    covered = len(waits) - n_keep
                    for j in range(0, covered, _MAXW):
                        _uid[0] += 1
                        carrier = mybir.InstNoOp(
                            name=f"waitsplit-{_uid[0]}",
                            sync_info=mybir.SyncInfo(
                                on_wait=waits[j : j + _MAXW], on_update=[]
                            ),
                            bass_nofuse=True,
                            engine=inst.engine,
                        )
                        nc.register_instruction(carrier, overwrite=True)
                        new.append(carrier)
                    si.on_wait = waits[covered:]
                new.append(inst)
            bb.instructions[:] = new


def _patched_tile_exit(self, exc_type, exc_val, exc_tb):
    r = _orig_tile_exit(self, exc_type, exc_val, exc_tb)
    if exc_type is None:
        _split_waits(self.nc)
    return r


if getattr(tile.TileContext.__exit__, "__name__", "") != "_patched_tile_exit":
    tile.TileContext.__exit__ = _patched_tile_exit

# ---------------------------------------------------------------------------
# Problem constants (hardcoded per spec)
# ---------------------------------------------------------------------------
B, V, D, H, ET = 32, 50, 6, 256, 4
T = ET - 1          # non-skipped edge types
E = V * (V - 1)     # 2450 real edges
EG = V * V          # 2500 grid edges (diagonal zero-weighted)
NCORES = 8
NB = B // NCORES    # samples per core
G = H // 128        # output-feature tiles (2)
K = H // 128        # contraction tiles (2)
CH = 512            # e-grid chunk = one PSUM bank
NCH = 5
EGP = CH * NCH      # 2560, padded grid (pad cols have ew == 0)

f32 = mybir.dt.float32
bf16 = mybir.dt.bfloat16
AF = mybir.ActivationFunctionType
ALU = mybir.AluOpType

_prog_cache = {}


def _build_program():
    nc = bass.Bass()

    def din(name, shape, dt):
        return nc.dram_tensor(name, list(shape), dt, kind="ExternalInput")

    hbf = din("hbf", (128, K, NB, V), bf16)
    hf32v = din("hf32", (128, K, NB, V), f32)
    xbf = din("xbf", (D, NB, V), bf16)
    xf32v = din("xf32", (D, NB, V), f32)
    ew = din("ew", (NB, T * EGP), bf16)
    w1 = din("w1", (128, K, T, 2, G, 128), bf16)
    rs = din("rs", (128, EGP), bf16)
    w2 = din("w2", (128, K, T, G, 128), bf16)
    wh = din("wh", (128, K, 3, G, 128), bf16)
    wi = din("wi", (D, 3, G, 128), bf16)
    wo12 = din("wo12", (128, K, 2, G, 128), bf16)
    wo3 = din("wo3", (128, K, D), bf16)
    b1d = din("bias1", (128, T, G), f32)
    b2d = din("bias2", (128, T, G), f32)
    bgd = din("biasg", (128, 3, G), f32)
    bod = din("biaso", (128, 2, G), f32)
    bo3d = din("bo3", (D, 1), f32)

    hnT = nc.dram_tensor("hnT", [128, K, NB, V], f32, kind="ExternalOutput")
    predT = nc.dram_tensor("predT", [D, NB, V], f32, kind="ExternalOutput")

    with tile.TileContext(nc) as tc:
        with (
            tc.tile_pool(name="singles", bufs=1) as singles,
            tc.tile_pool(name="work", bufs=2) as work,
            tc.tile_pool(name="m2pool", bufs=3) as m2pool,
            tc.tile_pool(name="psA", bufs=2, space="PSUM") as psA,
            tc.tile_pool(name="psB", bufs=1, space="PSUM") as psB,
        ):
            # ---------------- constants into SBUF ----------------
            # loads ordered by first use; late-needed ones go on the SWDGE
            # queue so the HWDGE queue isn't serialized in front of them
            def load(name, dram, shape, dt, eng=None):
                t = singles.tile(list(shape), dt, tag=name)
                (eng or nc.sync).dma_start(out=t, in_=dram[...])
                return t

            hbf_s = load("hbf", hbf, (128, K, NB, V), bf16)
            w1_s = singles.tile([128, K, T, 2, G, 128], bf16, tag="w1")
            nc.scalar.dma_start(out=w1_s[:, 0], in_=w1[:, 0])
            nc.sync.dma_start(out=w1_s[:, 1], in_=w1[:, 1])
            rs_s = load("rs", rs, (128, EGP), bf16, nc.scalar)
            b1_s = load("b1", b1d, (128, T, G), f32)
            w2_s = load("w2", w2, (128, K, T, G, 128), bf16)
            b2_s = load("b2", b2d, (128, T, G), f32)
            hf32_s = load("hf32", hf32v, (128, K, NB, V), f32, nc.gpsimd)
            xbf_s = load("xbf", xbf, (D, NB, V), bf16, nc.gpsimd)
            xf32_s = load("xf32", xf32v, (D, NB, V), f32, nc.gpsimd)
            wh_s = load("wh", wh, (128, K, 3, G, 128), bf16, nc.gpsimd)
            wi_s = load("wi", wi, (D, 3, G, 128), bf16, nc.gpsimd)
            wo12_s = load("wo12", wo12, (128, K, 2, G, 128), bf16, nc.gpsimd)
            wo3_s = load("wo3", wo3, (128, K, D), bf16, nc.gpsimd)
            bg_s = load("bg", bgd, (128, 3, G), f32, nc.gpsimd)
            bo_s = load("bo", bod, (128, 2, G), f32, nc.gpsimd)
            bo3_s = load("bo3", bo3d, (D, 1), f32, nc.gpsimd)

            agg_all = singles.tile([128, G, NB, V], f32, tag="agg")
            agg_bf = singles.tile([128, G, NB, V], bf16, tag="aggbf")
            hn_f = singles.tile([128, G, NB, V], f32, tag="hnf")
            hn_bf = singles.tile([128, G, NB, V], bf16, tag="hnbf")
            r_sb = singles.tile([128, G, NB, V], f32, tag="r_sb")
            i_sb = singles.tile([128, G, NB, V], f32, tag="i_sb")
            tmp = singles.tile([128, G, NB, V], f32, tag="tmp")
            n_sb = singles.tile([128, G, NB, V], f32, tag="n_sb")
            p1_bf = singles.tile([128, G, NB, V], bf16, tag="p1bf")
            p2_bf = singles.tile([128, G, NB, V], bf16, tag="p2bf")
            pred_sb = singles.tile([D, NB, V], f32, tag="pred")

            # ---------------- A/B node-level products (all samples) --------
            # A_vg[v,g] (rows 0-49) and B_vg[v,g] (rows 64-113) of the
            # selector-matmul stationary operand; evacuated via ACT while
            # the big constants are still streaming in.
            ab_sb = singles.tile([128, NB, T, H], bf16, tag="ab")
            nc.gpsimd.memset(
                ab_sb[32:64].rearrange("p b t g -> p (b t g)"), 0.0
            )
            nc.gpsimd.memset(
                ab_sb[96:128].rearrange("p b t g -> p (b t g)"), 0.0
            )
            for b in range(NB):
                pab = psA.tile([128, 3, CH], f32, tag="psa")
                for ab in range(2):
                    row0 = 0 if ab == 0 else 64
                    for t in range(T):
                        for k in range(K):
                            nc.tensor.matmul(
                                pab[row0 : row0 + V, t, 0:H],
                                hbf_s[:, k, b, :],
                                w1_s[:, k, t, ab].rearrange("p g m -> p (g m)"),
                                start=(k == 0),
                                stop=(k == K - 1),
                            )
                nc.vector.tensor_copy(
                    ab_sb[0:V, b].rearrange("p t g -> p (t g)"),
                    pab[0:V, 0:T, 0:H],
                )
                nc.vector.tensor_copy(
                    ab_sb[64 : 64 + V, b].rearrange("p t g -> p (t g)"),
                    pab[64 : 64 + V, 0:T, 0:H],
                )

            def emit_gru_mlp(b0, pspool, ptag):
                """GRU + output MLP for samples [b0, b0+1] (columns 2*V)."""
                bs = slice(b0, b0 + 2)
                NV2 = 2 * V
                nc.vector.tensor_copy(
                    agg_bf[:, :, bs, :], agg_all[:, :, bs, :]
                )
                xb = xbf_s[:, bs, :].rearrange("d b v -> d (b v)")
                for go in range(G):
                    pg = pspool.tile([128, 2, CH], f32, tag=ptag)
                    for gate in range(3):  # r, i, hh
                        for k in range(K):
                            nc.tensor.matmul(
                                pg[:, gate // 2, (gate % 2) * 128 : (gate % 2) * 128 + NV2],
                                wh_s[:, k, gate, go, :],
                                agg_bf[:, k, bs, :].rearrange("p b v -> p (b v)"),
                                start=(k == 0),
                                stop=(k == K - 1 and gate == 2),
                            )
                        if gate < 2:
                            nc.tensor.matmul(
                                pg[:, gate // 2, (gate % 2) * 128 : (gate % 2) * 128 + NV2],
                                wi_s[:, gate, go, :],
                                xb,
                                start=False,
                                stop=True,
                            )
                    # xn = Win @ inputs (chunk 1, cols 256:356)
                    nc.tensor.matmul(
                        pg[:, 1, 256 : 256 + NV2], wi_s[:, 2, go, :], xb,
                        start=True, stop=True,
                    )
                    rgo = r_sb[:, go, bs, :].rearrange("p b v -> p (b v)")
                    igo = i_sb[:, go, bs, :].rearrange("p b v -> p (b v)")
                    tgo = tmp[:, go, bs, :].rearrange("p b v -> p (b v)")
                    ngo = n_sb[:, go, bs, :].rearrange("p b v -> p (b v)")
                    nc.scalar.activation(
                        rgo, pg[:, 0, 0:NV2], AF.Sigmoid,
                        bias=bg_s[:, 0, go : go + 1],
                    )
                    nc.scalar.activation(
                        igo, pg[:, 0, 128 : 128 + NV2], AF.Sigmoid,
                        bias=bg_s[:, 1, go : go + 1],
                    )
                    nc.vector.tensor_mul(tgo, rgo, pg[:, 1, 0:NV2])
                    nc.vector.tensor_add(tgo, tgo, pg[:, 1, 256 : 256 + NV2])
                    nc.scalar.activation(
                        ngo, tgo, AF.Tanh, bias=bg_s[:, 2, go : go + 1]
                    )
                    # hidden_new = n + i*(hidden - n)
                    hgo = hf32_s[:, go, bs, :].rearrange("p b v -> p (b v)")
                    nc.vector.tensor_sub(tgo, hgo, ngo)
                    nc.vector.tensor_mul(tgo, igo, tgo)
                    nc.vector.tensor_add(
                        hn_f[:, go, bs, :].rearrange("p b v -> p (b v)"), ngo, tgo
                    )
                    nc.sync.dma_start(
                        out=hnT[:, go, bs], in_=hn_f[:, go, bs, :]
                    )
                    nc.vector.tensor_copy(hn_bf[:, go, bs, :], hn_f[:, go, bs, :])
                pm1 = pspool.tile([128, 2, CH], f32, tag=ptag)
                pm2 = pspool.tile([128, 2, CH], f32, tag=ptag)
                for go in range(G):
                    for k in range(K):
                        nc.tensor.matmul(
                            pm1[:, 0, go * 128 : go * 128 + NV2],
                            wo12_s[:, k, 0, go, :],
                            hn_bf[:, k, bs, :].rearrange("p b v -> p (b v)"),
                            start=(k == 0),
                            stop=(k == K - 1),
                        )
                    nc.vector.tensor_scalar(
                        p1_bf[:, go, bs, :].rearrange("p b v -> p (b v)"),
                        pm1[:, 0, go * 128 : go * 128 + NV2],
                        bo_s[:, 0, go : go + 1], 0.0,
                        op0=ALU.add, op1=ALU.max,
                    )
                for go in range(G):
                    for k in range(K):
                        nc.tensor.matmul(
                            pm2[:, 0, go * 128 : go * 128 + NV2],
                            wo12_s[:, k, 1, go, :],
                            p1_bf[:, k, bs, :].rearrange("p b v -> p (b v)"),
                            start=(k == 0),
                            stop=(k == K - 1),
                        )
                    nc.vector.tensor_scalar(
                        p2_bf[:, go, bs, :].rearrange("p b v -> p (b v)"),
                        pm2[:, 0, go * 128 : go * 128 + NV2],
                        bo_s[:, 1, go : go + 1], 0.0,
                        op0=ALU.add, op1=ALU.max,
                    )
                for k in range(K):
                    nc.tensor.matmul(
                        pm2[0:D, 1, 0:NV2],
                        wo3_s[:, k, :],
                        p2_bf[:, k, bs, :].rearrange("p b v -> p (b v)"),
                        start=(k == 0),
                        stop=(k == K - 1),
                    )
                nc.vector.scalar_tensor_tensor(
                    pred_sb[:, bs, :].rearrange("d b v -> d (b v)"),
                    pm2[0:D, 1, 0:NV2],
                    bo3_s[0:D, 0:1],
                    xf32_s[:, bs, :].rearrange("d b v -> d (b v)"),
                    op0=ALU.add,
                    op1=ALU.add,
                )
                nc.sync.dma_start(out=predT[:, bs], in_=pred_sb[:, bs, :])

            # ---------------- per-sample edge pipeline ----------------
            ew_tiles = {}
            for b in range(NB):
                ewt = work.tile([128, T, EGP], bf16, tag="ew")
                nc.gpsimd.dma_start(
                    out=ewt.rearrange("p t e -> p (t e)"),
                    in_=ew[b : b + 1, :].broadcast_to([128, T * EGP]),
                )
                ew_tiles[b] = ewt
                # m1[g, e] = tanh(A_vg[r(e), g] + B_vg[s(e), g] + b1[g]) via
                # a single-K-pass selector matmul (RS rows 50-63/114-127
                # are zero, killing the garbage lhsT rows).
                m1f = work.tile([128, K, T, EGP], bf16, tag="m1")
                for kh in range(K):
                    for t in range(T):
                        lhsT = ab_sb[:, b, t, kh * 128 : (kh + 1) * 128]
                        bias = b1_s[:, t, kh : kh + 1]
                        pa = psA.tile([128, 3, CH], f32, tag="psa")
                        for c in range(3):
                            nc.tensor.matmul(
                                pa[:, c, :],
                                lhsT,
                                rs_s[:, c * CH : (c + 1) * CH],
                                start=True,
                                stop=True,
                            )
                        nc.scalar.activation(
                            m1f[:, kh, t, 0 : 3 * CH],
                            pa[:, 0:3, :],
                            AF.Tanh,
                            bias=bias,
                        )
                        pb = psB.tile([128, 2, CH], f32, tag="psb")
                        for c in range(2):
                            nc.tensor.matmul(
                                pb[:, c, :],
                                lhsT,
                                rs_s[:, (3 + c) * CH : (4 + c) * CH],
                                start=True,
                                stop=True,
                            )
                        nc.scalar.activation(
                            m1f[:, kh, t, 3 * CH : EGP],
                            pb[:, 0:2, :],
                            AF.Tanh,
                            bias=bias,
                        )
                # m2 = tanh(m1 @ W2[t]^T + b2)
                for go in range(G):
                    m2 = m2pool.tile([128, T, EGP], bf16, tag="m2")
                    for t in range(T):
                        bias = b2_s[:, t, go : go + 1]
                        pa = psA.tile([128, 3, CH], f32, tag="psa")
                        for k in range(K):
                            for c in range(3):
                                nc.tensor.matmul(
                                    pa[:, c, :],
                                    w2_s[:, k, t, go, :],
                                    m1f[:, k, t, c * CH : (c + 1) * CH],
                                    start=(k == 0),
                                    stop=(k == K - 1),
                                )
                        nc.scalar.activation(
                            m2[:, t, 0 : 3 * CH], pa[:, 0:3, :], AF.Tanh, bias=bias
                        )
                        pb = psB.tile([128, 2, CH], f32, tag="psb")
                        for k in range(K):
                            for c in range(2):
                                nc.tensor.matmul(
                                    pb[:, c, :],
                                    w2_s[:, k, t, go, :],
                                    m1f[:, k, t, (3 + c) * CH : (4 + c) * CH],
                                    start=(k == 0),
                                    stop=(k == K - 1),
                                )
                        nc.scalar.activation(
                            m2[:, t, 3 * CH : EGP], pb[:, 0:2, :], AF.Tanh, bias=bias
                        )
                        # weight by edges as each type lands (keeps the DVE
                        # chain interleaved under the ACT evacuations)
                        nc.vector.tensor_mul(m2[:, t], m2[:, t], ewt[:, t])
                        if t == 1:
                            ps2 = work.tile([128, EGP], bf16, tag="ps2")
                            nc.vector.tensor_add(ps2, m2[:, 0], m2[:, 1])
                    nc.vector.tensor_add(ps2, ps2, m2[:, 2])
                    # aggregate over senders: agg[r] = sum_s ps2[r*V+s]
                    nc.vector.reduce_sum(
                        agg_all[:, go, b, :],
                        ps2[:, 0:EG].rearrange("p (r s) -> p r s", s=V),
                        axis=mybir.AxisListType.X,
                    )
            emit_gru_mlp(0, psB, "psb")
            emit_gru_mlp(2, psA, "psa")

    return nc


def _host_prep(inputs, hidden, edges, W1, b1, W2, b2, Whr, Whi, Whh,
               Wir, bir, Wii, bii, Win, bin_w, Wo1, bo1, Wo2, bo2, Wo3, bo3):
    """Build per-core input maps (all numpy, host-side layout only)."""
    asf = np.ascontiguousarray

    def b16(x):
        return asf(x.astype(ml_dtypes.bfloat16))

    # hidden^T: [p, k, b, v]
    hT = hidden.transpose(2, 0, 1).reshape(K, 128, B, V).transpose(1, 0, 2, 3)
    xT = inputs.transpose(2, 0, 1)  # [d, b, v]

    # edge-weight grid (recv-major), diag zero, scaled by 1/(T*(V-1)),
    # padded to EGP columns (pad cols zero)
    mask = np.ones((V, V)) - np.eye(V)
    send, recv = np.where(mask)
    M = np.zeros((B, V, V, T), np.float32)
    M[:, send, recv, :] = edges[:, :, 1:]
    ewg = M.transpose(0, 3, 2, 1).reshape(B, T, EG) / (T * (V - 1))
    ewp = np.zeros((B, T, EGP), np.float32)
    ewp[:, :, :EG] = ewg
    ewp = ewp.reshape(B, T * EGP)

    # weights as lhsT layouts
    W1a, W1b = W1[:, :, :H], W1[:, :, H:]

    def lhsT(wmat):  # [out, in] -> [p, k, g, m] with in = k*128+p, out = g*128+m
        return wmat.T.reshape(K, 128, G, 128).transpose(1, 0, 2, 3)

    w1_h = np.stack(
        [
            np.stack([lhsT(W1a[t]), lhsT(W1b[t])], axis=2)  # [p,k,ab,g,m]
            for t in range(T)
        ],
        axis=2,
    )  # [p, k, t, ab, g, m]
    w2_h = np.stack([lhsT(W2[t]) for t in range(T)], axis=2)  # [p,k,t,g,m]
    wh_h = np.stack([lhsT(Whr), lhsT(Whi), lhsT(Whh)], axis=2)
    wi_h = np.stack(
        [Wir.T.reshape(D, G, 128), Wii.T.reshape(D, G, 128),
         Win.T.reshape(D, G, 128)],
        axis=1,
    )  # [d, gate, g, m]
    wo12_h = np.stack([lhsT(Wo1), lhsT(Wo2)], axis=2)  # [p,k,o,g,m]
    wo3_h = Wo3.T.reshape(K, 128, D).transpose(1, 0, 2)  # [p,k,d]

    def pcol(vec):  # [H] -> [p, g]
        return vec.reshape(G, 128).T

    b1_h = np.stack([pcol(b1[t]) for t in range(T)], axis=1)  # [p,t,g]
    b2_h = np.stack([pcol(b2[t]) for t in range(T)], axis=1)
    bg_h = np.stack([pcol(bir), pcol(bii), pcol(bin_w)], axis=1)
    bo_h = np.stack([pcol(bo1), pcol(bo2)], axis=1)
    bo3_h = bo3.reshape(D, 1).astype(np.float32)

    # selector for the m1pre matmul: rows 0-49 pick recv (e // V), rows
    # 64-113 pick send (e % V); all other rows and pad columns zero.
    rs_h = np.zeros((128, EGP), np.float32)
    e = np.arange(EG)
    rs_h[e // V, e] = 1.0
    rs_h[64 + e % V, e] = 1.0

    shared = {
        "rs": b16(rs_h),
        "w1": b16(w1_h), "w2": b16(w2_h), "wh": b16(wh_h), "wi": b16(wi_h),
        "wo12": b16(wo12_h), "wo3": b16(wo3_h),
        "bias1": asf(b1_h.astype(np.float32)),
        "bias2": asf(b2_h.astype(np.float32)),
        "biasg": asf(bg_h.astype(np.float32)),
        "biaso": asf(bo_h.astype(np.float32)),
        "bo3": bo3_h,
    }
    in_maps = []
    for c in range(NCORES):
        sl = slice(c * NB, (c + 1) * NB)
        m = dict(shared)
        m["hbf"] = b16(hT[:, :, sl])
        m["hf32"] = asf(hT[:, :, sl].astype(np.float32))
        m["xbf"] = b16(xT[:, sl])
        m["xf32"] = asf(xT[:, sl].astype(np.float32))
        m["ew"] = b16(ewp[sl])
        in_maps.append(m)
    return in_maps


def kernel(**inputs):
    if "prog" not in _prog_cache:
        _prog_cache["prog"] = _build_program()
    nc = _prog_cache["prog"]

    in_maps = _host_prep(**{k: np.asarray(v) for k, v in inputs.items()})
    res = run_bass_kernel_spmd(nc, in_maps, list(range(NCORES)))

    pred = np.empty((B, V, D), np.float32)
    hidden_new = np.empty((B, V, H), np.float32)
    for c in range(NCORES):
        sl = slice(c * NB, (c + 1) * NB)
        hnT = np.asarray(res.results[c]["hnT"], np.float32)  # [p,k,b,v]
        predT = np.asarray(res.results[c]["predT"], np.float32)  # [d,b,v]
        hidden_new[sl] = hnT.transpose(2, 3, 1, 0).reshape(NB, V, H)
        pred[sl] = predT.transpose(1, 2, 0)
    return pred, hidden_new


# revision 44
# speedup vs baseline: 1.1602x; 1.0048x over previous
"""DNRI message-passing step on 8 Trainium2 NeuronCores.

Strategy: data-parallel over batch B=32 -> 4 samples per core; params
replicated. Per sample, the per-edge first-layer matmul is factorized into
per-node products A_vg = hidden @ W1a^T, B_vg = hidden @ W1b^T, and the
dense 50x50 (recv-major) edge grid is produced by a single-K-pass
"selector" matmul whose stationary operand stacks [A_vg; B_vg] and whose
moving operand is a constant 0/1 recv/send selector — the PE does the
edge-grid gather/expansion for free. Only the second edge-MLP layer runs
at edge granularity. Edge aggregation is a contiguous strided reduction
over the send axis (no gather/scatter). All heavy matmuls in bf16 with
fp32 PSUM accumulation; GRU/output MLP batched over the 4 samples.

The edge grid is padded 2500 -> 2560 so every PSUM chunk is a full
512-col bank; each PSUM group is a 3-bank tile (double-buffered pool)
plus a 2-bank tile, so matmuls of group i+1 overlap the ACT tanh
evacuation of group i and the pipeline stays ACT-paced (~ACT busy 75%+
of kernel time). Measured ~171 us/core on TRN2 at rel err ~7e-4.
"""

import numpy as np
import ml_dtypes

import concourse.bass as bass
import concourse.mybir as mybir
import concourse.tile as tile
from concourse.bass_utils import run_bass_kernel_spmd

# ----------------------------------------------------------------------------
# Workaround for walrus "Too many sync wait commands": the neuronxcc in this
# container accepts at most one semaphore wait per instruction, but Tile's
# sem-assignment can emit several. After scheduling, hoist excess waits onto
# same-engine InstNoOp carriers inserted immediately before the instruction.
# ----------------------------------------------------------------------------
_MAXW = 1
_orig_tile_exit = tile.TileContext.__exit__
_uid = [0]


def _split_waits(nc):
    for func in nc.m.functions:
        for bb in func.blocks:
            new = []
            for inst in bb.instructions:
                si = inst.sync_info
                waits = list(si.on_wait) if si and si.on_wait else []
                if len(waits) > _MAXW:
                    n_keep = len(waits) % _MAXW or _MAXW
                    covered = len(waits) - n_keep
                    for j in range(0, covered, _MAXW):
                        _uid[0] += 1
                        carrier = mybir.InstNoOp(
                            name=f"waitsplit-{_uid[0]}",
                            sync_info=mybir.SyncInfo(
                                on_wait=waits[j : j + _MAXW], on_update=[]
                            ),
                            bass_nofuse=True,
                            engine=inst.engine,
                        )
                        nc.register_instruction(carrier, overwrite=True)
                        new.append(carrier)
                    si.on_wait = waits[covered:]
                new.append(inst)
            bb.instructions[:] = new


def _patched_tile_exit(self, exc_type, exc_val, exc_tb):
    r = _orig_tile_exit(self, exc_type, exc_val, exc_tb)
    if exc_type is None:
        _split_waits(self.nc)
    return r


if getattr(tile.TileContext.__exit__, "__name__", "") != "_patched_tile_exit":
    tile.TileContext.__exit__ = _patched_tile_exit

# ---------------------------------------------------------------------------
# Problem constants (hardcoded per spec)
# ---------------------------------------------------------------------------
B, V, D, H, ET = 32, 50, 6, 256, 4
T = ET - 1          # non-skipped edge types
E = V * (V - 1)     # 2450 real edges
EG = V * V          # 2500 grid edges (diagonal zero-weighted)
NCORES = 8
NB = B // NCORES    # samples per core
G = H // 128        # output-feature tiles (2)
K = H // 128        # contraction tiles (2)
CH = 512            # e-grid chunk = one PSUM bank
NCH = 5
EGP = CH * NCH      # 2560, padded grid (pad cols have ew == 0)

f32 = mybir.dt.float32
bf16 = mybir.dt.bfloat16
AF = mybir.ActivationFunctionType
ALU = mybir.AluOpType

_prog_cache = {}


def _build_program():
    nc = bass.Bass()

    def din(name, shape, dt):
        return nc.dram_tensor(name, list(shape), dt, kind="ExternalInput")

    hbf = din("hbf", (128, K, NB, V), bf16)
    hf32v = din("hf32", (128, K, NB, V), f32)
    xbf = din("xbf", (D, NB, V), bf16)
    xf32v = din("xf32", (D, NB, V), f32)
    ew = din("ew", (NB, T * EGP), bf16)
    w1 = din("w1", (128, K, T, 2, G, 128), bf16)
    rs = din("rs", (128, EGP), bf16)
    w2 = din("w2", (128, K, T, G, 128), bf16)
    wh = din("wh", (128, K, 3, G, 128), bf16)
    wi = din("wi", (D, 3, G, 128), bf16)
    wo12 = din("wo12", (128, K, 2, G, 128), bf16)
    wo3 = din("wo3", (128, K, D), bf16)
    b1d = din("bias1", (128, T, G), f32)
    b2d = din("bias2", (128, T, G), f32)
    bgd = din("biasg", (128, 3, G), f32)
    bod = din("biaso", (128, 2, G), f32)
    bo3d = din("bo3", (D, 1), f32)

    hnT = nc.dram_tensor("hnT", [128, K, NB, V], f32, kind="ExternalOutput")
    predT = nc.dram_tensor("predT", [D, NB, V], f32, kind="ExternalOutput")

    with tile.TileContext(nc) as tc:
        with (
            tc.tile_pool(name="singles", bufs=1) as singles,
            tc.tile_pool(name="work", bufs=2) as work,
            tc.tile_pool(name="m2pool", bufs=3) as m2pool,
            tc.tile_pool(name="psA", bufs=2, space="PSUM") as psA,
            tc.tile_pool(name="psB", bufs=1, space="PSUM") as psB,
        ):
            # ---------------- constants into SBUF ----------------
            # loads ordered by first use; late-needed ones go on the SWDGE
            # queue so the HWDGE queue isn't serialized in front of them
            def load(name, dram, shape, dt, eng=None):
                t = singles.tile(list(shape), dt, tag=name)
                (eng or nc.sync).dma_start(out=t, in_=dram[...])
                return t

            hbf_s = load("hbf", hbf, (128, K, NB, V), bf16)
            w1_s = singles.tile([128, K, T, 2, G, 128], bf16, tag="w1")
            nc.scalar.dma_start(out=w1_s[:, 0], in_=w1[:, 0])
            nc.sync.dma_start(out=w1_s[:, 1], in_=w1[:, 1])
            rs_s = load("rs", rs, (128, EGP), bf16, nc.scalar)
            b1_s = load("b1", b1d, (128, T, G), f32)
            w2_s = load("w2", w2, (128, K, T, G, 128), bf16)
            b2_s = load("b2", b2d, (128, T, G), f32)
            hf32_s = load("hf32", hf32v, (128, K, NB, V), f32, nc.gpsimd)
            xbf_s = load("xbf", xbf, (D, NB, V), bf16, nc.gpsimd)
            xf32_s = load("xf32", xf32v, (D, NB, V), f32, nc.gpsimd)
            wh_s = load("wh", wh, (128, K, 3, G, 128), bf16, nc.gpsimd)
            wi_s = load("wi", wi, (D, 3, G, 128), bf16, nc.gpsimd)
            wo12_s = load("wo12", wo12, (128, K, 2, G, 128), bf16, nc.gpsimd)
            wo3_s = load("wo3", wo3, (128, K, D), bf16, nc.gpsimd)
            bg_s = load("bg", bgd, (128, 3, G), f32, nc.gpsimd)
            bo_s = load("bo", bod, (128, 2, G), f32, nc.gpsimd)
            bo3_s = load("bo3", bo3d, (D, 1), f32, nc.gpsimd)

            agg_all = singles.tile([128, G, NB, V], f32, tag="agg")
            agg_bf = singles.tile([128, G, NB, V], bf16, tag="aggbf")
            hn_f = singles.tile([128, G, NB, V], f32, tag="hnf")
            hn_bf = singles.tile([128, G, NB, V], bf16, tag="hnbf")
            r_sb = singles.tile([128, G, NB, V], f32, tag="r_sb")
            i_sb = singles.tile([128, G, NB, V], f32, tag="i_sb")
            tmp = singles.tile([128, G, NB, V], f32, tag="tmp")
            n_sb = singles.tile([128, G, NB, V], f32, tag="n_sb")
            p1_bf = singles.tile([128, G, NB, V], bf16, tag="p1bf")
            p2_bf = singles.tile([128, G, NB, V], bf16, tag="p2bf")
            pred_sb = singles.tile([D, NB, V], f32, tag="pred")

            # ---------------- A/B node-level products (all samples) --------
            # A_vg[v,g] (rows 0-49) and B_vg[v,g] (rows 64-113) of the
            # selector-matmul stationary operand; evacuated via ACT while
            # the big constants are still streaming in.
            ab_sb = singles.tile([128, NB, T, H], bf16, tag="ab")
            nc.gpsimd.memset(
                ab_sb[32:64].rearrange("p b t g -> p (b t g)"), 0.0
            )
            nc.gpsimd.memset(
                ab_sb[96:128].rearrange("p b t g -> p (b t g)"), 0.0
            )
            for b in range(NB):
                pab = psA.tile([128, 3, CH], f32, tag="psa")
                for ab in range(2):
                    row0 = 0 if ab == 0 else 64
                    for t in range(T):
                        for k in range(K):
                            nc.tensor.matmul(
                                pab[row0 : row0 + V, t, 0:H],
                                hbf_s[:, k, b, :],
                                w1_s[:, k, t, ab].rearrange("p g m -> p (g m)"),
                                start=(k == 0),
                                stop=(k == K - 1),
                            )
                nc.vector.tensor_copy(
                    ab_sb[0:V, b].rearrange("p t g -> p (t g)"),
                    pab[0:V, 0:T, 0:H],
                )
                nc.vector.tensor_copy(
                    ab_sb[64 : 64 + V, b].rearrange("p t g -> p (t g)"),
                    pab[64 : 64 + V, 0:T, 0:H],
                )

            def emit_gru_mlp(b0, pspool, ptag):
                """GRU + output MLP for samples [b0, b0+1] (columns 2*V)."""
                bs = slice(b0, b0 + 2)
                NV2 = 2 * V
                nc.vector.tensor_copy(
                    agg_bf[:, :, bs, :], agg_all[:, :, bs, :]
                )
                xb = xbf_s[:, bs, :].rearrange("d b v -> d (b v)")
                for go in range(G):
                    pg = pspool.tile([128, 2, CH], f32, tag=ptag)
                    for gate in range(3):  # r, i, hh
                        for k in range(K):
                            nc.tensor.matmul(
                                pg[:, gate // 2, (gate % 2) * 128 : (gate % 2) * 128 + NV2],
                                wh_s[:, k, gate, go, :],
                                agg_bf[:, k, bs, :].rearrange("p b v -> p (b v)"),
                                start=(k == 0),
                                stop=(k == K - 1 and gate == 2),
                            )
                        if gate < 2:
                            nc.tensor.matmul(
                                pg[:, gate // 2, (gate % 2) * 128 : (gate % 2) * 128 + NV2],
                                wi_s[:, gate, go, :],
                                xb,
                                start=False,
                                stop=True,
                            )
                    # xn = Win @ inputs (chunk 1, cols 256:356)
                    nc.tensor.matmul(
                        pg[:, 1, 256 : 256 + NV2], wi_s[:, 2, go, :], xb,
                        start=True, stop=True,
                    )
                    rgo = r_sb[:, go, bs, :].rearrange("p b v -> p (b v)")
                    igo = i_sb[:, go, bs, :].rearrange("p b v -> p (b v)")
                    tgo = tmp[:, go, bs, :].rearrange("p b v -> p (b v)")
                    ngo = n_sb[:, go, bs, :].rearrange("p b v -> p (b v)")
                    nc.scalar.activation(
                        rgo, pg[:, 0, 0:NV2], AF.Sigmoid,
                        bias=bg_s[:, 0, go : go + 1],
                    )
                    nc.scalar.activation(
                        igo, pg[:, 0, 128 : 128 + NV2], AF.Sigmoid,
                        bias=bg_s[:, 1, go : go + 1],
                    )
                    nc.vector.tensor_mul(tgo, rgo, pg[:, 1, 0:NV2])
                    nc.vector.tensor_add(tgo, tgo, pg[:, 1, 256 : 256 + NV2])
                    nc.scalar.activation(
                        ngo, tgo, AF.Tanh, bias=bg_s[:, 2, go : go + 1]
                    )
                    # hidden_new = n + i*(hidden - n)
                    hgo = hf32_s[:, go, bs, :].rearrange("p b v -> p (b v)")
                    nc.vector.tensor_sub(tgo, hgo, ngo)
                    nc.vector.tensor_mul(tgo, igo, tgo)
                    nc.vector.tensor_add(
                        hn_f[:, go, bs, :].rearrange("p b v -> p (b v)"), ngo, tgo
                    )
                    nc.sync.dma_start(
                        out=hnT[:, go, bs], in_=hn_f[:, go, bs, :]
                    )
                    nc.vector.tensor_copy(hn_bf[:, go, bs, :], hn_f[:, go, bs, :])
                pm1 = pspool.tile([128, 2, CH], f32, tag=ptag)
                pm2 = pspool.tile([128, 2, CH], f32, tag=ptag)
                for go in range(G):
                    for k in range(K):
                        nc.tensor.matmul(
                            pm1[:, 0, go * 128 : go * 128 + NV2],
                            wo12_s[:, k, 0, go, :],
                            hn_bf[:, k, bs, :].rearrange("p b v -> p (b v)"),
                            start=(k == 0),
                            stop=(k == K - 1),
                        )
                    nc.vector.tensor_scalar(
                        p1_bf[:, go, bs, :].rearrange("p b v -> p (b v)"),
                        pm1[:, 0, go * 128 : go * 128 + NV2],
                        bo_s[:, 0, go : go + 1], 0.0,
                        op0=ALU.add, op1=ALU.max,
                    )
                for go in range(G):
                    for k in range(K):
                        nc.tensor.matmul(
                            pm2[:, 0, go * 128 : go * 128 + NV2],
                            wo12_s[:, k, 1, go, :],
                            p1_bf[:, k, bs, :].rearrange("p b v -> p (b v)"),
                            start=(k == 0),
                            stop=(k == K - 1),
                        )
                    nc.vector.tensor_scalar(
                        p2_bf[:, go, bs, :].rearrange("p b v -> p (b v)"),
                        pm2[:, 0, go * 128 : go * 128 + NV2],
                        bo_s[:, 1, go : go + 1], 0.0,
                        op0=ALU.add, op1=ALU.max,
                    )
                for k in range(K):
                    nc.tensor.matmul(
                        pm2[0:D, 1, 0:NV2],
                        wo3_s[:, k, :],
                        p2_bf[:, k, bs, :].rearrange("p b v -> p (b v)"),
                        start=(k == 0),
                        stop=(k == K - 1),
                    )
                nc.vector.scalar_tensor_tensor(
                    pred_sb[:, bs, :].rearrange("d b v -> d (b v)"),
                    pm2[0:D, 1, 0:NV2],
                    bo3_s[0:D, 0:1],
                    xf32_s[:, bs, :].rearrange("d b v -> d (b v)"),
                    op0=ALU.add,
                    op1=ALU.add,
                )
                nc.sync.dma_start(out=predT[:, bs], in_=pred_sb[:, bs, :])

            # ---------------- per-sample edge pipeline ----------------
            ew_tiles = {}
            for b in range(NB):
                ewt = work.tile([128, T, EGP], bf16, tag="ew")
                nc.gpsimd.dma_start(
                    out=ewt.rearrange("p t e -> p (t e)"),
                    in_=ew[b : b + 1, :].broadcast_to([128, T * EGP]),
                )
                ew_tiles[b] = ewt
                # m1[g, e] = tanh(A_vg[r(e), g] + B_vg[s(e), g] + b1[g]) via
                # a single-K-pass selector matmul (RS rows 50-63/114-127
                # are zero, killing the garbage lhsT rows).
                m1f = work.tile([128, K, T, EGP], bf16, tag="m1")
                for kh in range(K):
                    for t in range(T):
                        lhsT = ab_sb[:, b, t, kh * 128 : (kh + 1) * 128]
                        bias = b1_s[:, t, kh : kh + 1]
                        pa = psA.tile([128, 3, CH], f32, tag="psa")
                        for c in range(3):
                            nc.tensor.matmul(
                                pa[:, c, :],
                                lhsT,
                                rs_s[:, c * CH : (c + 1) * CH],
                                start=True,
                                stop=True,
                            )
                        nc.scalar.activation(
                            m1f[:, kh, t, 0 : 3 * CH],
                            pa[:, 0:3, :],
                            AF.Tanh,
                            bias=bias,
                        )
                        pb = psB.tile([128, 2, CH], f32, tag="psb")
                        for c in range(2):
                            nc.tensor.matmul(
                                pb[:, c, :],
                                lhsT,
                                rs_s[:, (3 + c) * CH : (4 + c) * CH],
                                start=True,
                                stop=True,
                            )
                        nc.scalar.activation(
                            m1f[:, kh, t, 3 * CH : EGP],
                            pb[:, 0:2, :],
                            AF.Tanh,
                            bias=bias,
                        )
                # m2 = tanh(m1 @ W2[t]^T + b2)
                for go in range(G):
                    m2 = m2pool.tile([128, T, EGP], bf16, tag="m2")
                    for t in range(T):
                        bias = b2_s[:, t, go : go + 1]
                        pa = psA.tile([128, 3, CH], f32, tag="psa")
                        for k in range(K):
                            for c in range(3):
                                nc.tensor.matmul(
                                    pa[:, c, :],
                                    w2_s[:, k, t, go, :],
                                    m1f[:, k, t, c * CH : (c + 1) * CH],
                                    start=(k == 0),
                                    stop=(k == K - 1),
                                )
                        nc.scalar.activation(
                            m2[:, t, 0 : 3 * CH], pa[:, 0:3, :], AF.Tanh, bias=bias
                        )
                        pb = psB.tile([128, 2, CH], f32, tag="psb")
                        for k in range(K):
                            for c in range(2):
                                nc.tensor.matmul(
                                    pb[:, c, :],
                                    w2_s[:, k, t, go, :],
                                    m1f[:, k, t, (3 + c) * CH : (4 + c) * CH],
                                    start=(k == 0),
                                    stop=(k == K - 1),
                                )
                        nc.scalar.activation(
                            m2[:, t, 3 * CH : EGP], pb[:, 0:2, :], AF.Tanh, bias=bias
                        )
                        # weight by edges as each type lands (keeps the DVE
                        # chain interleaved under the ACT evacuations)
                        nc.vector.tensor_mul(m2[:, t], m2[:, t], ewt[:, t])
                        if t == 1:
                            ps2 = work.tile([128, EGP], bf16, tag="ps2")
                            nc.vector.tensor_add(ps2, m2[:, 0], m2[:, 1])
                    nc.vector.tensor_add(ps2, ps2, m2[:, 2])
                    # aggregate over senders: agg[r] = sum_s ps2[r*V+s]
                    nc.vector.reduce_sum(
                        agg_all[:, go, b, :],
                        ps2[:, 0:EG].rearrange("p (r s) -> p r s", s=V),
                        axis=mybir.AxisListType.X,
                    )
            emit_gru_mlp(0, psB, "psb")
            emit_gru_mlp(2, psA, "psa")

    return nc


def _host_prep(inputs, hidden, edges, W1, b1, W2, b2, Whr, Whi, Whh,
               Wir, bir, Wii, bii, Win, bin_w, Wo1, bo1, Wo2, bo2, Wo3, bo3):
    """Build per-core input maps (all numpy, host-side layout only)."""
    asf = np.ascontiguousarray

    def b16(x):
        return asf(x.astype(ml_dtypes.bfloat16))

    # hidden^T: [p, k, b, v]
    hT = hidden.transpose(2, 0, 1).reshape(K, 128, B, V).transpose(1, 0, 2, 3)
    xT = inputs.transpose(2, 0, 1)  # [d, b, v]

    # edge-weight grid (recv-major), diag zero, scaled by 1/(T*(V-1)),
    # padded to EGP columns (pad cols zero)
    mask = np.ones((V, V)) - np.eye(V)
    send, recv = np.where(mask)
    M = np.zeros((B, V, V, T), np.float32)
    M[:, send, recv, :] = edges[:, :, 1:]
    ewg = M.transpose(0, 3, 2, 1).reshape(B, T, EG) / (T * (V - 1))
    ewp = np.zeros((B, T, EGP), np.float32)
    ewp[:, :, :EG] = ewg
    ewp = ewp.reshape(B, T * EGP)

    # weights as lhsT layouts
    W1a, W1b = W1[:, :, :H], W1[:, :, H:]

    def lhsT(wmat):  # [out, in] -> [p, k, g, m] with in = k*128+p, out = g*128+m
        return wmat.T.reshape(K, 128, G, 128).transpose(1, 0, 2, 3)

    w1_h = np.stack(
        [
            np.stack([lhsT(W1a[t]), lhsT(W1b[t])], axis=2)  # [p,k,ab,g,m]
            for t in range(T)
        ],
        axis=2,
    )  # [p, k, t, ab, g, m]
    w2_h = np.stack([lhsT(W2[t]) for t in range(T)], axis=2)  # [p,k,t,g,m]
    wh_h = np.stack([lhsT(Whr), lhsT(Whi), lhsT(Whh)], axis=2)
    wi_h = np.stack(
        [Wir.T.reshape(D, G, 128), Wii.T.reshape(D, G, 128),
         Win.T.reshape(D, G, 128)],
        axis=1,
    )  # [d, gate, g, m]
    wo12_h = np.stack([lhsT(Wo1), lhsT(Wo2)], axis=2)  # [p,k,o,g,m]
    wo3_h = Wo3.T.reshape(K, 128, D).transpose(1, 0, 2)  # [p,k,d]

    def pcol(vec):  # [H] -> [p, g]
        return vec.reshape(G, 128).T

    b1_h = np.stack([pcol(b1[t]) for t in range(T)], axis=1)  # [p,t,g]
    b2_h = np.stack([pcol(b2[t]) for t in range(T)], axis=1)
    bg_h = np.stack([pcol(bir), pcol(bii), pcol(bin_w)], axis=1)
    bo_h = np.stack([pcol(bo1), pcol(bo2)], axis=1)
    bo3_h = bo3.reshape(D, 1).astype(np.float32)

    # selector for the m1pre matmul: rows 0-49 pick recv (e // V), rows
    # 64-113 pick send (e % V); all other rows and pad columns zero.
    rs_h = np.zeros((128, EGP), np.float32)
    e = np.arange(EG)
    rs_h[e // V, e] = 1.0
    rs_h[64 + e % V, e] = 1.0

    shared = {
        "rs": b16(rs_h),
        "w1": b16(w1_h), "w2": b16(w2_h), "wh": b16(wh_h), "wi": b16(wi_h),
        "wo12": b16(wo12_h), "wo3": b16(wo3_h),
        "bias1": asf(b1_h.astype(np.float32)),
        "bias2": asf(b2_h.astype(np.float32)),
        "biasg": asf(bg_h.astype(np.float32)),
        "biaso": asf(bo_h.astype(np.float32)),
        "bo3": bo3_h,
    }
    in_maps = []
    for c in range(NCORES):
        sl = slice(c * NB, (c + 1) * NB)
        m = dict(shared)
        m["hbf"] = b16(hT[:, :, sl])
        m["hf32"] = asf(hT[:, :, sl].astype(np.float32))
        m["xbf"] = b16(xT[:, sl])
        m["xf32"] = asf(xT[:, sl].astype(np.float32))
        m["ew"] = b16(ewp[sl])
        in_maps.append(m)
    return in_maps


def kernel(**inputs):
    if "prog" not in _prog_cache:
        _prog_cache["prog"] = _build_program()
    nc = _prog_cache["prog"]

    in_maps = _host_prep(**{k: np.asarray(v) for k, v in inputs.items()})
    res = run_bass_kernel_spmd(nc, in_maps, list(range(NCORES)))

    pred = np.empty((B, V, D), np.float32)
    hidden_new = np.empty((B, V, H), np.float32)
    for c in range(NCORES):
        sl = slice(c * NB, (c + 1) * NB)
        hnT = np.asarray(res.results[c]["hnT"], np.float32)  # [p,k,b,v]
        predT = np.asarray(res.results[c]["predT"], np.float32)  # [d,b,v]
        hidden_new[sl] = hnT.transpose(2, 3, 1, 0).reshape(NB, V, H)
        pred[sl] = predT.transpose(1, 2, 0)
    return pred, hidden_new
